# revision 1
# baseline (speedup 1.0000x reference)
"""KLayerHeteroRGCN on 8 trn2 NeuronCores via Bass/Tile.

Strategy (hardcoded for N=50000, R=4, E=800000, D=128):
- Host: bucket+sort edges by destination node owner core / 128-node dst tile,
  compute degree normalizers, build per-edge gather indices and one-hot
  metadata. All float tensor compute happens on device.
- Device (per layer l in 0..2):
  Phase A: y_r = dout_r * (x @ W_r) for all nodes (replicated on all cores),
    written to a combined gather table Y [R*NP, 128] in DRAM.
  Phase B (per 128-node dst tile owned by this core): indirect-DMA gather of
    y rows for each 128-edge block, one-hot matmul segment-sum into PSUM
    (din folded into the one-hot values), then bias + L2-normalize +
    leaky-relu epilogue.
  Between layers: AllGather of the per-core node features.
- The final update_all(copy_u,sum) + mean_nodes round collapses to a
  weighted column sum: sum_n outdeg_total[n] * h3[n] (computed on device as
  a matmul against the out-degree vector, accumulated across tiles).
- Host: sum 8 partial [128] vectors, /N, @Wlin + blin, sigmoid.
"""
import os
import sys
import numpy as np

sys.path.insert(0, "/opt/trn_rl_repo")

N = 50000
R = 4
E = 800000
D = 128
C = 8
P = 128
NLOC = N // C          # 6250 dst nodes per core
T = 49                 # dst tiles per core (6272 = 49*128 padded)
TP = T * P             # 6272
N0P = 391 * P          # 50048: padded rows of layer-0 x
N12P = C * TP          # 50176: rows of all-gathered h (per-core 6272 blocks)

LAST_EXEC_NS = None
LAST_RESULTS = None


def _host_prep(feat, src, dst, W1, b1, W2, b2, W3, b3):
    f32 = np.float32
    srcl = src.astype(np.int64)
    dstl = dst.astype(np.int64)
    deg_out = np.stack([np.maximum(np.bincount(srcl[r], minlength=N), 1) for r in range(R)]).astype(f32)
    deg_in = np.stack([np.maximum(np.bincount(dstl[r], minlength=N), 1) for r in range(R)]).astype(f32)
    dout = deg_out ** -0.5   # [R, N]
    din = deg_in ** -0.5     # [R, N]

    nodes = np.arange(N, dtype=np.int64)
    g = (nodes // NLOC) * TP + (nodes % NLOC)   # global node -> row in all-gathered h

    srcf = srcl.reshape(-1)
    dstf = dstl.reshape(-1)
    relf = np.repeat(np.arange(R, dtype=np.int64), E)
    owner = dstf // NLOC
    dloc_all = dstf - owner * NLOC
    tile_all = dloc_all // P
    ct = owner * T + tile_all
    counts = np.bincount(ct, minlength=C * T)
    B = int(np.ceil(counts.max() / P))
    S = B * P

    gidx0 = np.zeros((C, T, S), np.int32)
    gidx12 = np.zeros((C, T, S), np.int32)
    dlocf = np.full((C, T, S), 255.0, f32)
    alpha = np.zeros((C, T, S), f32)

    order = np.argsort(ct, kind="stable")
    grp_start = np.zeros(C * T, np.int64)
    grp_start[1:] = np.cumsum(counts)[:-1]
    pos = np.arange(order.size, dtype=np.int64) - grp_start[ct[order]]
    es = order
    c_s = owner[es]
    t_s = tile_all[es]
    gidx0[c_s, t_s, pos] = (relf[es] * N0P + srcf[es]).astype(np.int32)
    gidx12[c_s, t_s, pos] = (relf[es] * N12P + g[srcf[es]]).astype(np.int32)
    dlocf[c_s, t_s, pos] = (dloc_all[es] % P).astype(f32)
    alpha[c_s, t_s, pos] = din[relf[es], dstf[es]]

    # [C,T,S] -> [C,T,P,B]: block b of tile t sits at [:, :, :, b]
    gidx0 = np.ascontiguousarray(gidx0.reshape(C, T, B, P).transpose(0, 1, 3, 2))
    gidx12 = np.ascontiguousarray(gidx12.reshape(C, T, B, P).transpose(0, 1, 3, 2))
    dlocf = np.ascontiguousarray(dlocf.reshape(C, T, B, P).transpose(0, 1, 3, 2))
    alpha = np.ascontiguousarray(alpha.reshape(C, T, B, P).transpose(0, 1, 3, 2))

    douts0 = np.zeros((N0P, R), f32)
    douts0[:N, :] = dout.T
    douts12 = np.zeros((N12P, R), f32)
    douts12[g, :] = dout.T

    wcnt = np.zeros(N, np.int64)
    for r in range(R):
        wcnt += np.bincount(srcl[r], minlength=N)
    wpool = np.zeros((C, TP, 1), f32)
    wpool[nodes // NLOC, nodes % NLOC, 0] = wcnt.astype(f32)

    Wcat = np.stack([np.ascontiguousarray(Wl.transpose(1, 0, 2).reshape(D, R * D))
                     for Wl in (W1, W2, W3)]).astype(f32)
    bsum = np.stack([np.tile(bl.sum(0), (P, 1)) for bl in (b1, b2, b3)]).astype(f32)
    iota = np.tile(np.arange(P, dtype=f32), (P, 1))
    featp = np.zeros((N0P, D), f32)
    featp[:N] = feat

    common = dict(featp=featp, Wcat=Wcat, bsum=bsum, iota=iota,
                  douts0=douts0, douts12=douts12)
    percore = [dict(gidx0=gidx0[c], gidx12=gidx12[c], dlocf=dlocf[c],
                    alpha=alpha[c], wpool=wpool[c]) for c in range(C)]
    return B, common, percore


def _build(B):
    import concourse.bass as bass
    import concourse.bacc as bacc
    import concourse.tile as tile
    from concourse import mybir
    from concourse.bass import IndirectOffsetOnAxis
    from concourse.masks import make_identity

    dt = mybir.dt
    f32 = dt.float32
    Alu = mybir.AluOpType
    Act = mybir.ActivationFunctionType

    nc = bacc.Bacc("TRN2", target_bir_lowering=False, debug=False, num_devices=C)

    def inp(name, shape, d=f32):
        return nc.dram_tensor(name, list(shape), d, kind="ExternalInput").ap()

    feat_t = inp("featp", (N0P, D))
    Wcat_t = inp("Wcat", (3, D, R * D))
    bsum_t = inp("bsum", (3, P, P))
    iota_t = inp("iota", (P, P))
    douts0_t = inp("douts0", (N0P, R))
    douts12_t = inp("douts12", (N12P, R))
    gidx0_t = inp("gidx0", (T, P, B), dt.int32)
    gidx12_t = inp("gidx12", (T, P, B), dt.int32)
    dloc_t = inp("dlocf", (T, P, B))
    alpha_t = inp("alpha", (T, P, B))
    wpool_t = inp("wpool", (TP, 1))
    out_t = nc.dram_tensor("pooled", [P, 1], f32, kind="ExternalOutput").ap()

    with tile.TileContext(nc) as tc:
        with tc.tile_pool(name="dram", bufs=1, space="DRAM") as dp, \
             tc.tile_pool(name="const", bufs=1) as cp, \
             tc.tile_pool(name="pa", bufs=4) as pa, \
             tc.tile_pool(name="paps", bufs=2, space="PSUM") as paps, \
             tc.tile_pool(name="xtps", bufs=2, space="PSUM") as xtps, \
             tc.tile_pool(name="pb", bufs=3) as pb, \
             tc.tile_pool(name="gath", bufs=12) as gp, \
             tc.tile_pool(name="pbps", bufs=3, space="PSUM") as pbps, \
             tc.tile_pool(name="plps", bufs=1, space="PSUM") as plps:

            Y0 = dp.tile([R * N0P, D], f32, name="Y0", tag="Y0")
            Y1 = dp.tile([R * N12P, D], f32, name="Y1", tag="Y1")
            Y2 = dp.tile([R * N12P, D], f32, name="Y2", tag="Y2")
            hp0 = dp.tile([TP, D], f32, name="hp0", tag="hp0")
            hp1 = dp.tile([TP, D], f32, name="hp1", tag="hp1")
            hf0 = dp.tile([N12P, D], f32, name="hf0", tag="hf0", addr_space="Shared")
            hf1 = dp.tile([N12P, D], f32, name="hf1", tag="hf1", addr_space="Shared")
            Ys = (Y0, Y1, Y2)

            iota_s = cp.tile([P, P], f32, name="iota_s")
            nc.sync.dma_start(out=iota_s[:], in_=iota_t[:, :])
            ident = cp.tile([P, P], f32, name="ident")
            make_identity(nc, ident[:])
            pacc = cp.tile([P, 1], f32, name="pacc")
            nc.vector.memset(pacc[:], 0.0)

            for l in range(3):
                xsrc = feat_t if l == 0 else (hf0 if l == 1 else hf1)[:]
                Yl = Ys[l]
                NP = N0P if l == 0 else N12P
                nt = NP // P
                dsrc = douts0_t if l == 0 else douts12_t
                gsrc = gidx0_t if l == 0 else gidx12_t

                W_s = cp.tile([P, R * D], f32, name=f"W_s{l}", tag=f"W_s{l}")
                nc.sync.dma_start(out=W_s[:], in_=Wcat_t[l])
                bs_s = cp.tile([P, P], f32, name=f"bs_s{l}", tag=f"bs_s{l}")
                nc.sync.dma_start(out=bs_s[:], in_=bsum_t[l])

                # ---- Phase A: Y[r*NP + n] = dout_r[n] * (x @ W_r)[n] ----
                for i in range(nt):
                    xt = pa.tile([P, P], f32, tag="xt", name=f"xt_{l}_{i}")
                    nc.sync.dma_start(out=xt[:], in_=xsrc[i * P:(i + 1) * P, :])
                    xT_ps = xtps.tile([P, P], f32, tag="xT_ps", name=f"xTp_{l}_{i}")
                    nc.tensor.transpose(out=xT_ps[:], in_=xt[:], identity=ident[:])
                    xT = pa.tile([P, P], f32, tag="xT", name=f"xT_{l}_{i}")
                    nc.scalar.activation(out=xT[:], in_=xT_ps[:], func=Act.Copy)
                    do4 = pa.tile([P, R], f32, tag="do4", name=f"do4_{l}_{i}")
                    nc.sync.dma_start(out=do4[:], in_=dsrc[i * P:(i + 1) * P, :])
                    z = paps.tile([P, R * D], f32, tag="z", name=f"z_{l}_{i}")
                    nc.tensor.matmul(out=z[:], lhsT=xT[:], rhs=W_s[:], start=True, stop=True)
                    ys = pa.tile([P, R * D], f32, tag="ys", name=f"ys_{l}_{i}")
                    nc.vector.tensor_tensor(
                        out=ys[:].rearrange("p (r d) -> p r d", d=D),
                        in0=z[:].rearrange("p (r d) -> p r d", d=D),
                        in1=do4[:].unsqueeze(2).to_broadcast([P, R, D]),
                        op=Alu.mult)
                    for r in range(R):
                        nc.sync.dma_start(
                            out=Yl[r * NP + i * P: r * NP + (i + 1) * P, :],
                            in_=ys[:, r * D:(r + 1) * D])

                # ---- Phase B: per dst tile gather + one-hot matmul segment sum ----
                for t in range(T):
                    idx = pb.tile([P, B], dt.int32, tag="idx", name=f"idx_{l}_{t}")
                    nc.sync.dma_start(out=idx[:], in_=gsrc[t])
                    dl = pb.tile([P, B], f32, tag="dl", name=f"dl_{l}_{t}")
                    nc.sync.dma_start(out=dl[:], in_=dloc_t[t])
                    al = pb.tile([P, B], f32, tag="al", name=f"al_{l}_{t}")
                    nc.sync.dma_start(out=al[:], in_=alpha_t[t])
                    oh = pb.tile([P, B * P], f32, tag="oh", name=f"oh_{l}_{t}")
                    oh3 = oh[:].rearrange("p (b j) -> p b j", j=P)
                    nc.vector.tensor_tensor(
                        out=oh3,
                        in0=dl[:].unsqueeze(2).to_broadcast([P, B, P]),
                        in1=iota_s[:].unsqueeze(1).to_broadcast([P, B, P]),
                        op=Alu.is_equal)
                    nc.vector.tensor_tensor(
                        out=oh3, in0=oh3,
                        in1=al[:].unsqueeze(2).to_broadcast([P, B, P]),
                        op=Alu.mult)
                    agg = pbps.tile([P, P], f32, tag="agg", name=f"agg_{l}_{t}")
                    for b in range(B):
                        gt = gp.tile([P, P], f32, tag="gt", name=f"gt_{l}_{t}_{b}")
                        nc.gpsimd.indirect_dma_start(
                            out=gt[:], out_offset=None, in_=Yl[:, :],
                            in_offset=IndirectOffsetOnAxis(ap=idx[:, b:b + 1], axis=0))
                        nc.tensor.matmul(out=agg[:], lhsT=oh[:, b * P:(b + 1) * P],
                                         rhs=gt[:], start=(b == 0), stop=(b == B - 1))
                    # epilogue: bias (+ l2-normalize + leaky-relu on layers 0/1 only)
                    hpre = pb.tile([P, P], f32, tag="hpre", name=f"hpre_{l}_{t}")
                    nc.vector.tensor_tensor(out=hpre[:], in0=agg[:], in1=bs_s[:], op=Alu.add)
                    if l < 2:
                        scr = pb.tile([P, P], f32, tag="scr", name=f"scr_{l}_{t}")
                        rsq = pb.tile([P, 1], f32, tag="rsq", name=f"rsq_{l}_{t}")
                        nc.scalar.activation(out=scr[:], in_=hpre[:], func=Act.Square,
                                             accum_out=rsq[:])
                        nrm = pb.tile([P, 1], f32, tag="nrm", name=f"nrm_{l}_{t}")
                        nc.scalar.sqrt(nrm[:], rsq[:])
                        nrm2 = pb.tile([P, 1], f32, tag="nrm2", name=f"nrm2_{l}_{t}")
                        nc.vector.tensor_scalar_max(nrm2[:], nrm[:], 1e-12)
                        inv = pb.tile([P, 1], f32, tag="inv", name=f"inv_{l}_{t}")
                        nc.vector.reciprocal(inv[:], nrm2[:])
                        hn = pb.tile([P, P], f32, tag="hn", name=f"hn_{l}_{t}")
                        nc.vector.tensor_scalar(out=hn[:], in0=hpre[:], scalar1=inv[:, :1],
                                                scalar2=None, op0=Alu.mult)
                        ng = pb.tile([P, P], f32, tag="ng", name=f"ng_{l}_{t}")
                        nc.scalar.mul(ng[:], hn[:], 0.01)
                        ho = pb.tile([P, P], f32, tag="ho", name=f"ho_{l}_{t}")
                        nc.vector.tensor_tensor(out=ho[:], in0=hn[:], in1=ng[:], op=Alu.max)
                        hp = hp0 if l == 0 else hp1
                        nc.sync.dma_start(out=hp[t * P:(t + 1) * P, :], in_=ho[:])
                    else:
                        ho = hpre
                        wt = pb.tile([P, 1], f32, tag="wt", name=f"wt_{t}")
                        nc.sync.dma_start(out=wt[:], in_=wpool_t[t * P:(t + 1) * P, :])
                        pp = plps.tile([P, 1], f32, tag="pp", name=f"pp_{t}")
                        nc.tensor.matmul(out=pp[:], lhsT=ho[:], rhs=wt[:],
                                         start=True, stop=True)
                        nc.vector.tensor_tensor(out=pacc[:], in0=pacc[:], in1=pp[:],
                                                op=Alu.add)
                if l < 2:
                    hp, hf = (hp0, hf0) if l == 0 else (hp1, hf1)
                    nc.gpsimd.collective_compute(
                        "AllGather", Alu.bypass,
                        replica_groups=[list(range(C))],
                        ins=[hp[:].opt()], outs=[hf[:].opt()])

            nc.sync.dma_start(out=out_t[:, :], in_=pacc[:])

    nc.compile()
    return nc


def _time_exec(nc, in_maps, iters=3):
    """Warm-run timing of the compiled NEFF via PJRT with inputs pre-staged
    on device (mirrors bass2jax.run_bass_via_pjrt's multi-core path)."""
    import time
    import jax
    import numpy as jnp_np
    from jax.sharding import Mesh, PartitionSpec, NamedSharding
    from jax.experimental.shard_map import shard_map
    from concourse import bass2jax, mybir

    bass2jax.install_neuronx_cc_hook()
    in_names, out_names, out_avals, zero_outs = [], [], [], []
    for alloc in nc.m.functions[0].allocations:
        if not isinstance(alloc, mybir.MemoryLocationSet):
            continue
        name = alloc.memorylocations[0].name
        pname = nc.partition_id_tensor.name if nc.partition_id_tensor else None
        if alloc.kind == "ExternalInput":
            if name != pname:
                in_names.append(name)
        elif alloc.kind == "ExternalOutput":
            out_names.append(name)
            shape = tuple(alloc.tensor_shape)
            dtype = mybir.dt.np(alloc.dtype)
            out_avals.append(jax.core.ShapedArray(shape, dtype))
            zero_outs.append(np.zeros(shape, dtype))
    n_params = len(in_names)
    pname = nc.partition_id_tensor.name if nc.partition_id_tensor else None
    all_names = in_names + out_names + ([pname] if pname else [])

    def _body(*args):
        operands = list(args)
        if pname is not None:
            operands.append(bass2jax.partition_id_tensor())
        outs = bass2jax._bass_exec_p.bind(
            *operands, out_avals=tuple(out_avals), in_names=tuple(all_names),
            out_names=tuple(out_names), lowering_input_output_aliases=(),
            sim_require_finite=True, sim_require_nnan=True, nc=nc)
        return tuple(outs)

    devices = jax.devices()[:C]
    mesh = Mesh(np.asarray(devices), ("core",))
    spec = PartitionSpec("core")
    n_outs = len(out_names)
    sharded = jax.jit(
        shard_map(_body, mesh=mesh, in_specs=(spec,) * (n_params + n_outs),
                  out_specs=(spec,) * n_outs, check_rep=False),
        keep_unused=True)
    sh = NamedSharding(mesh, spec)
    concat_in = [jax.device_put(
        np.concatenate([np.asarray(m[name]) for m in in_maps], axis=0), sh)
        for name in in_names]
    concat_zero = [jax.device_put(
        np.zeros((C * z.shape[0], *z.shape[1:]), z.dtype), sh) for z in zero_outs]
    out = sharded(*concat_in, *concat_zero)   # warmup + compile
    jax.block_until_ready(out)
    best = None
    for _ in range(iters):
        t0 = time.perf_counter()
        out = sharded(*concat_in, *concat_zero)
        jax.block_until_ready(out)
        dt_ns = (time.perf_counter() - t0) * 1e9
        best = dt_ns if best is None else min(best, dt_ns)
    return int(best)


def kernel(feat, src, dst, W1, b1, W2, b2, W3, b3, Wlin, blin):
    global LAST_EXEC_NS, LAST_RESULTS
    feat = np.asarray(feat, np.float32)
    src = np.asarray(src, np.int32)
    dst = np.asarray(dst, np.int32)
    W1, b1 = np.asarray(W1, np.float32), np.asarray(b1, np.float32)
    W2, b2 = np.asarray(W2, np.float32), np.asarray(b2, np.float32)
    W3, b3 = np.asarray(W3, np.float32), np.asarray(b3, np.float32)
    Wlin, blin = np.asarray(Wlin, np.float32), np.asarray(blin, np.float32)

    B, common, percore = _host_prep(feat, src, dst, W1, b1, W2, b2, W3, b3)
    nc = _build(B)

    from concourse.bass_utils import run_bass_kernel_spmd
    in_maps = [dict(common, **percore[c]) for c in range(C)]
    res = run_bass_kernel_spmd(nc, in_maps, core_ids=list(range(C)))
    LAST_RESULTS = res
    if os.environ.get("KTIME"):
        LAST_EXEC_NS = _time_exec(nc, in_maps)

    total = np.zeros(D, np.float64)
    for c in range(C):
        total += res.results[c]["pooled"][:, 0].astype(np.float64)
    hg = (total / N).astype(np.float32)
    out = hg @ Wlin + blin
    return (1.0 / (1.0 + np.exp(-out.astype(np.float64)))).astype(np.float32)[None, :]



# revision 13
# speedup vs baseline: 1.3740x; 1.3740x over previous
"""KLayerHeteroRGCN on 8 trn2 NeuronCores via Bass/Tile.

Strategy (hardcoded for N=50000, R=4, E=800000, D=128):
- Aggregate-then-transform: per relation r, agg_r = scatter(alpha_e * x[src_e])
  with alpha = dout[r,src]*din[r,dst] folded per edge; then
  h = sum_r agg_r @ W_r + sum_r b_r. This removes the per-relation Y tables
  of the transform-first formulation (no [R*N, D] writes, 1 gather per edge).
- All node tables are bf16 in a g-mapped layout [50176, 128] shared by all 3
  layers (g[n] = (n//6250)*6272 + n%6250), so one index structure serves
  every layer. Layer 0 gathers from host-prepped featg; layers 1-2 from the
  AllGathered h tables.
- Gathers use bulk gpsimd.dma_gather (int16 indices, 16-partition wrapped and
  replicated across the 8 Q7 groups). Since indices are int16, each dst tile
  issues two gathers: rows [0,32768) of the table and rows [32768,50176).
- Per dst tile (128 dst nodes): blocks of 128 edges, grouped per (rel, half).
  One-hot (alpha at column dloc) built on DVE in bf16, then per block one
  matmul aggT_r[f,dst] += gt_b^T @ oh_b accumulated in PSUM; 4 small matmuls
  vs W_r produce h[dst,hid]; bias + L2norm + leaky-relu epilogue.
- Final layer: pooled = sum_n outdeg_total[n]*h3[n] via matmul with the
  out-degree vector; host sums cores, /N, @Wlin+blin, sigmoid.
"""
import os
import sys
import numpy as np

sys.path.insert(0, "/opt/trn_rl_repo")

N = 50000
R = 4
E = 800000
D = 128
C = 8
P = 128
NLOC = N // C          # 6250 dst nodes per core
T = 49                 # dst tiles per core (6272 = 49*128 padded)
TP = T * P             # 6272
NG = C * TP            # 50176 rows of the g-mapped node table
LO = 32768             # int16 gather index limit; rows >= LO use the hi view
HI = NG - LO           # 17408

LAST_EXEC_NS = None
LAST_RESULTS = None


def _host_prep(feat, src, dst, W1, b1, W2, b2, W3, b3):
    """Returns shared (across cores) per-tile block structure + host tensors.
    One program serves all 8 cores, so the per-tile group block counts are
    the elementwise max over cores; each core's stream is padded (idx=0,
    dl=255, alpha=0) to that shape."""
    import ml_dtypes
    f32 = np.float32
    bf16 = ml_dtypes.bfloat16
    srcl = src.astype(np.int64).reshape(-1)
    dstl = dst.astype(np.int64).reshape(-1)
    relf = np.repeat(np.arange(R, dtype=np.int64), E)

    deg_out = np.stack([np.maximum(np.bincount(srcl[r * E:(r + 1) * E], minlength=N), 1)
                        for r in range(R)]).astype(f32)
    deg_in = np.stack([np.maximum(np.bincount(dstl[r * E:(r + 1) * E], minlength=N), 1)
                       for r in range(R)]).astype(f32)
    dout = deg_out ** -0.5
    din = deg_in ** -0.5

    nodes = np.arange(N, dtype=np.int64)
    g = (nodes // NLOC) * TP + (nodes % NLOC)

    gsrc = g[srcl]
    half = (gsrc >= LO).astype(np.int64)
    idxval = gsrc - LO * half
    owner = dstl // NLOC
    dloc_all = dstl % NLOC
    tile_all = dloc_all // P
    dpos = dloc_all % P
    alpha = (dout[relf, srcl] * din[relf, dstl]).astype(f32)

    # group key: (owner, tile, half, rel) -- lo groups (all rels) precede hi
    key = (((owner * T + tile_all) * 2 + half) * R + relf)
    NKEY = C * T * 2 * R
    counts = np.bincount(key, minlength=NKEY).reshape(C, T, 2 * R)

    blk_shared = (-(-counts // P)).max(axis=0)   # [T, 2R] per-tile max over cores
    empty = blk_shared.sum(axis=1) == 0
    blk_shared[empty, 0] = 1                     # guarantee >=1 block per tile
    tb_t = blk_shared.sum(axis=1)                # [T]
    TBMAX = int(tb_t.max())
    BLOMAX = int(blk_shared[:, :R].sum(1).max())
    BHIMAX = int(blk_shared[:, R:].sum(1).max())

    # slot base of each group within its tile stream (units of edges)
    gbase = np.zeros((T, 2 * R), np.int64)
    gbase[:, 1:] = np.cumsum(blk_shared[:, :-1], axis=1)
    gbase_e = gbase * P

    order = np.argsort(key, kind="stable")
    grp_start = np.zeros(NKEY, np.int64)
    grp_start[1:] = np.cumsum(counts.reshape(-1))[:-1]
    pos_in_grp = np.arange(order.size, dtype=np.int64) - grp_start[key[order]]
    es = order
    c_s = owner[es]
    t_s = tile_all[es]
    grp_s = key[es] % (2 * R)
    slot = gbase_e[t_s, grp_s] + pos_in_grp      # slot within tile stream

    idx_st = np.zeros((C, T, TBMAX * P), np.int16)
    dl_st = np.full((C, T, TBMAX * P), 255.0, f32)
    al_st = np.zeros((C, T, TBMAX * P), f32)
    idx_st[c_s, t_s, slot] = idxval[es].astype(np.int16)
    dl_st[c_s, t_s, slot] = dpos[es].astype(f32)
    al_st[c_s, t_s, slot] = alpha[es]

    # idx wrapped layout: stream pos i -> [i%16, i//16], replicated to 8 groups
    idx_w = idx_st.reshape(C, T, TBMAX * 8, 16).transpose(0, 1, 3, 2)  # [C,T,16,W]
    idx_hw = np.ascontiguousarray(np.tile(idx_w, (1, 1, 8, 1)))        # [C,T,128,W]

    # dl/alpha: [C,T,128,TBMAX] with [p, b] = stream slot b*128+p
    dla = np.empty((C, T, P, 2 * TBMAX), f32)
    dla[:, :, :, :TBMAX] = dl_st.reshape(C, T, TBMAX, P).transpose(0, 1, 3, 2)
    dla[:, :, :, TBMAX:] = al_st.reshape(C, T, TBMAX, P).transpose(0, 1, 3, 2)
    dla = dla.astype(bf16)

    featg = np.zeros((NG, D), f32)
    featg[g, :] = feat
    featg = featg.astype(bf16)

    wcnt = np.zeros(N, np.int64)
    for r in range(R):
        wcnt += np.bincount(srcl[r * E:(r + 1) * E], minlength=N)
    wpool = np.zeros((C, TP, 1), f32)
    wpool[nodes // NLOC, nodes % NLOC, 0] = wcnt.astype(f32)

    Wc = np.stack([np.ascontiguousarray(Wl.transpose(1, 0, 2).reshape(D, R * D))
                   for Wl in (W1, W2, W3)]).astype(bf16)
    bsum = np.stack([np.tile(bl.sum(0), (P, 1)) for bl in (b1, b2, b3)]).astype(f32)
    iota = np.tile(np.arange(P, dtype=f32), (P, 1)).astype(bf16)

    common = dict(featg=featg, Wc=Wc, bsum=bsum, iota=iota)
    percore = [dict(gidx=idx_hw[c], dla=dla[c], wpool=wpool[c]) for c in range(C)]
    return TBMAX, BLOMAX, BHIMAX, blk_shared, common, percore


def _build(TBMAX, BLOMAX, BHIMAX, blk_list):
    import concourse.bass as bass
    import concourse.bacc as bacc
    import concourse.tile as tile
    from concourse import mybir

    dt = mybir.dt
    f32 = dt.float32
    bf16 = dt.bfloat16
    Alu = mybir.AluOpType
    Act = mybir.ActivationFunctionType

    # 32KB/partition descriptor carveout: one dma_gather call is limited by
    # SWDGE ring capacity; empirically 1024 idxs (65 descs/engine) per call is
    # safe at 32KB with tile's 3-deep SWDGE pipelining (960 crashes at 16KB).
    nc = bacc.Bacc("TRN2", target_bir_lowering=False, debug=False, num_devices=C,
                   dynamic_dma_scratch_size=32768)

    def inp(name, shape, d=f32):
        return nc.dram_tensor(name, list(shape), d, kind="ExternalInput").ap()

    WMAX = TBMAX * 8
    featg_t = inp("featg", (NG, D), bf16)
    Wc_t = inp("Wc", (3, D, R * D), bf16)
    bsum_t = inp("bsum", (3, P, P))
    iota_t = inp("iota", (P, P), bf16)
    gidx_t = inp("gidx", (T, P, WMAX), dt.int16)
    dla_t = inp("dla", (T, P, 2 * TBMAX), bf16)
    wpool_t = inp("wpool", (TP, 1))
    out_t = nc.dram_tensor("pooled", [P, 1], f32, kind="ExternalOutput").ap()

    # blk structure is identical across cores by construction of maxima, but
    # per (tile) the counts differ; they are shared across cores only in shape.
    # blk_list here is the per-core list for THIS compile: all cores share one
    # compiled program, so we use the per-tile max over cores for the loop
    # structure and mask the difference via alpha=0 padding.
    with tile.TileContext(nc) as tc:
        with tc.tile_pool(name="dram", bufs=1, space="DRAM") as dp, \
             tc.tile_pool(name="const", bufs=1) as cp, \
             tc.tile_pool(name="pin", bufs=3) as pin, \
             tc.tile_pool(name="glo", bufs=3) as glo, \
             tc.tile_pool(name="ghi", bufs=3) as ghi, \
             tc.tile_pool(name="poh", bufs=3) as poh, \
             tc.tile_pool(name="aggps", bufs=2, space="PSUM") as aggps, \
             tc.tile_pool(name="hps", bufs=2, space="PSUM") as hps, \
             tc.tile_pool(name="pe", bufs=3) as pe, \
             tc.tile_pool(name="plps", bufs=1, space="PSUM") as plps:

            if os.environ.get("DBG_HP"):
                hp0 = nc.dram_tensor("hp0dbg", [TP, D], bf16, kind="ExternalOutput").ap()
            else:
                hp0 = dp.tile([TP, D], bf16, name="hp0", tag="hp0")
            hp1 = dp.tile([TP, D], bf16, name="hp1", tag="hp1")
            hf0 = dp.tile([NG, D], bf16, name="hf0", tag="hf0", addr_space="Shared")
            hf1 = dp.tile([NG, D], bf16, name="hf1", tag="hf1", addr_space="Shared")

            iota_s = cp.tile([P, P], bf16, name="iota_s")
            nc.sync.dma_start(out=iota_s[:], in_=iota_t[:, :])
            pacc = cp.tile([P, 1], f32, name="pacc")
            nc.vector.memset(pacc[:], 0.0)
            W_s = cp.tile([P, 3 * R * D], bf16, name="W_s")
            for l in range(3):
                nc.sync.dma_start(out=W_s[:, l * R * D:(l + 1) * R * D], in_=Wc_t[l])
            bs_s = cp.tile([P, 3 * P], f32, name="bs_s")
            for l in range(3):
                nc.sync.dma_start(out=bs_s[:, l * P:(l + 1) * P], in_=bsum_t[l])

            NL = int(os.environ.get("NLAYERS", "3"))
            for l in range(NL):
                table = featg_t if l == 0 else (hf0 if l == 1 else hf1)[:]
                for t in range(T):
                    blks = blk_list[t]            # [8]: (lo r0..r3, hi r0..r3)
                    nlo = int(blks[:R].sum())
                    nhi = int(blks[R:].sum())
                    tb = nlo + nhi
                    idx = pin.tile([P, WMAX], dt.int16, tag="idx", name=f"idx_{l}_{t}")
                    nc.sync.dma_start(out=idx[:, :8 * tb], in_=gidx_t[t, :, :8 * tb])
                    dla = pin.tile([P, 2 * TBMAX], bf16, tag="dla", name=f"dla_{l}_{t}")
                    nc.sync.dma_start(out=dla[:], in_=dla_t[t])

                    CH = 8   # max blocks (1024 idxs) per dma_gather call
                    glo_t = glo.tile([P, BLOMAX * P], bf16, tag="glo", name=f"glo_{l}_{t}")
                    glo3 = glo_t[:].rearrange("p (b f) -> p b f", f=P)
                    for c0 in range(0, nlo, CH):
                        c1 = min(c0 + CH, nlo)
                        nc.gpsimd.dma_gather(
                            out_ap=glo3[:, c0:c1, :],
                            in_ap=table[0:LO, :],
                            idxs_ap=idx[:, 8 * c0:8 * c1],
                            num_idxs=(c1 - c0) * P, num_idxs_reg=(c1 - c0) * P,
                            elem_size=D)
                    if nhi:
                        ghi_t = ghi.tile([P, BHIMAX * P], bf16, tag="ghi", name=f"ghi_{l}_{t}")
                        ghi3 = ghi_t[:].rearrange("p (b f) -> p b f", f=P)
                        for c0 in range(0, nhi, CH):
                            c1 = min(c0 + CH, nhi)
                            nc.gpsimd.dma_gather(
                                out_ap=ghi3[:, c0:c1, :],
                                in_ap=table[LO:NG, :],
                                idxs_ap=idx[:, 8 * (nlo + c0):8 * (nlo + c1)],
                                num_idxs=(c1 - c0) * P, num_idxs_reg=(c1 - c0) * P,
                                elem_size=D)

                    oh = poh.tile([P, TBMAX * P], bf16, tag="oh", name=f"oh_{l}_{t}")
                    oh3 = oh[:, :tb * P].rearrange("p (b j) -> p b j", j=P)
                    nc.vector.tensor_tensor(
                        out=oh3,
                        in0=dla[:, :tb].unsqueeze(2).to_broadcast([P, tb, P]),
                        in1=iota_s[:].unsqueeze(1).to_broadcast([P, tb, P]),
                        op=Alu.is_equal)
                    nc.vector.tensor_tensor(
                        out=oh3, in0=oh3,
                        in1=dla[:, TBMAX:TBMAX + tb].unsqueeze(2).to_broadcast([P, tb, P]),
                        op=Alu.mult)

                    aggT = aggps.tile([P, R * P], f32, tag="aggT", name=f"aggT_{l}_{t}")
                    # stream block index of each group: groups 0..3 lo, 4..7 hi
                    gstart = np.concatenate([[0], np.cumsum(blks)[:-1]])
                    first = {}
                    # PE accumulation groups must be contiguous in issue order:
                    # issue per rel (lo blocks then hi blocks of that rel).
                    for r in range(R):
                        rblocks = [int(gstart[r]) + i for i in range(int(blks[r]))] + \
                                  [int(gstart[R + r]) + i for i in range(int(blks[R + r]))]
                        if not rblocks:
                            continue
                        first[r] = True
                        for k, b0 in enumerate(rblocks):
                            gsrc_t = glo_t if b0 < nlo else ghi_t
                            gb = b0 if b0 < nlo else b0 - nlo
                            nc.tensor.matmul(
                                out=aggT[:, r * P:(r + 1) * P],
                                lhsT=gsrc_t[:, gb * P:(gb + 1) * P],
                                rhs=oh[:, b0 * P:(b0 + 1) * P],
                                start=(k == 0), stop=(k == len(rblocks) - 1))

                    aggs = pe.tile([P, R * P], bf16, tag="aggs", name=f"aggs_{l}_{t}")
                    nc.scalar.activation(out=aggs[:], in_=aggT[:], func=Act.Copy)
                    if l == 0 and t == 0 and os.environ.get("DBG_T0"):
                        dglo = nc.dram_tensor("dbg_glo", [P, BLOMAX * P], bf16,
                                              kind="ExternalOutput").ap()
                        nc.sync.dma_start(out=dglo[:, :], in_=glo_t[:])
                        dghi = nc.dram_tensor("dbg_ghi", [P, BHIMAX * P], bf16,
                                              kind="ExternalOutput").ap()
                        nc.sync.dma_start(out=dghi[:, :], in_=ghi_t[:])
                        doh = nc.dram_tensor("dbg_oh", [P, TBMAX * P], bf16,
                                             kind="ExternalOutput").ap()
                        nc.sync.dma_start(out=doh[:, :], in_=oh[:])
                        dagg = nc.dram_tensor("dbg_aggs", [P, R * P], bf16,
                                              kind="ExternalOutput").ap()
                        nc.sync.dma_start(out=dagg[:, :], in_=aggs[:])
                    h = hps.tile([P, P], f32, tag="h", name=f"h_{l}_{t}")
                    present = sorted(first)
                    for r in present:
                        nc.tensor.matmul(
                            out=h[:],
                            lhsT=aggs[:, r * P:(r + 1) * P],
                            rhs=W_s[:, (l * R + r) * D:(l * R + r + 1) * D],
                            start=(r == present[0]), stop=(r == present[-1]))
                    hpre = pe.tile([P, P], f32, tag="hpre", name=f"hpre_{l}_{t}")
                    nc.vector.tensor_tensor(out=hpre[:], in0=h[:],
                                            in1=bs_s[:, l * P:(l + 1) * P], op=Alu.add)
                    if l < 2:
                        scr = pe.tile([P, P], f32, tag="scr", name=f"scr_{l}_{t}")
                        rsq = pe.tile([P, 1], f32, tag="rsq", name=f"rsq_{l}_{t}")
                        nc.scalar.activation(out=scr[:], in_=hpre[:], func=Act.Square,
                                             accum_out=rsq[:])
                        nrm = pe.tile([P, 1], f32, tag="nrm", name=f"nrm_{l}_{t}")
                        nc.scalar.sqrt(nrm[:], rsq[:])
                        nrm2 = pe.tile([P, 1], f32, tag="nrm2", name=f"nrm2_{l}_{t}")
                        nc.vector.tensor_scalar_max(nrm2[:], nrm[:], 1e-12)
                        inv = pe.tile([P, 1], f32, tag="inv", name=f"inv_{l}_{t}")
                        nc.vector.reciprocal(inv[:], nrm2[:])
                        hn = pe.tile([P, P], f32, tag="hn", name=f"hn_{l}_{t}")
                        nc.vector.tensor_scalar(out=hn[:], in0=hpre[:], scalar1=inv[:, :1],
                                                scalar2=None, op0=Alu.mult)
                        ng = pe.tile([P, P], f32, tag="ng", name=f"ng_{l}_{t}")
                        nc.scalar.mul(ng[:], hn[:], 0.01)
                        ho = pe.tile([P, P], bf16, tag="ho", name=f"ho_{l}_{t}")
                        nc.vector.tensor_tensor(out=ho[:], in0=hn[:], in1=ng[:], op=Alu.max)
                        hp = hp0 if l == 0 else hp1
                        nc.scalar.dma_start(out=hp[t * P:(t + 1) * P, :], in_=ho[:])
                    else:
                        wt = pe.tile([P, 1], f32, tag="wt", name=f"wt_{t}")
                        nc.sync.dma_start(out=wt[:], in_=wpool_t[t * P:(t + 1) * P, :])
                        pp = plps.tile([P, 1], f32, tag="pp", name=f"pp_{t}")
                        nc.tensor.matmul(out=pp[:], lhsT=hpre[:], rhs=wt[:],
                                         start=True, stop=True)
                        nc.vector.tensor_tensor(out=pacc[:], in0=pacc[:], in1=pp[:],
                                                op=Alu.add)
                if l < 2 and l + 1 < NL:
                    hp, hf = (hp0, hf0) if l == 0 else (hp1, hf1)
                    nc.gpsimd.collective_compute(
                        "AllGather", Alu.bypass,
                        replica_groups=[list(range(C))],
                        ins=[hp[:].opt()], outs=[hf[:].opt()])

            nc.sync.dma_start(out=out_t[:, :], in_=pacc[:])

    nc.compile()
    return nc


def _time_exec(nc, in_maps, iters=3):
    """Warm-run timing of the compiled NEFF via PJRT with inputs pre-staged
    on device (mirrors bass2jax.run_bass_via_pjrt's multi-core path)."""
    import time
    import jax
    from jax.sharding import Mesh, PartitionSpec, NamedSharding
    from jax.experimental.shard_map import shard_map
    from concourse import bass2jax, mybir

    bass2jax.install_neuronx_cc_hook()
    in_names, out_names, out_avals, zero_outs = [], [], [], []
    for alloc in nc.m.functions[0].allocations:
        if not isinstance(alloc, mybir.MemoryLocationSet):
            continue
        name = alloc.memorylocations[0].name
        pname = nc.partition_id_tensor.name if nc.partition_id_tensor else None
        if alloc.kind == "ExternalInput":
            if name != pname:
                in_names.append(name)
        elif alloc.kind == "ExternalOutput":
            out_names.append(name)
            shape = tuple(alloc.tensor_shape)
            dtype = mybir.dt.np(alloc.dtype)
            out_avals.append(jax.core.ShapedArray(shape, dtype))
            zero_outs.append(np.zeros(shape, dtype))
    n_params = len(in_names)
    pname = nc.partition_id_tensor.name if nc.partition_id_tensor else None
    all_names = in_names + out_names + ([pname] if pname else [])

    def _body(*args):
        operands = list(args)
        if pname is not None:
            operands.append(bass2jax.partition_id_tensor())
        outs = bass2jax._bass_exec_p.bind(
            *operands, out_avals=tuple(out_avals), in_names=tuple(all_names),
            out_names=tuple(out_names), lowering_input_output_aliases=(),
            sim_require_finite=True, sim_require_nnan=True, nc=nc)
        return tuple(outs)

    devices = jax.devices()[:C]
    mesh = Mesh(np.asarray(devices), ("core",))
    spec = PartitionSpec("core")
    n_outs = len(out_names)
    sharded = jax.jit(
        shard_map(_body, mesh=mesh, in_specs=(spec,) * (n_params + n_outs),
                  out_specs=(spec,) * n_outs, check_rep=False),
        keep_unused=True)
    sh = NamedSharding(mesh, spec)
    concat_in = [jax.device_put(
        np.concatenate([np.asarray(m[name]) for m in in_maps], axis=0), sh)
        for name in in_names]
    concat_zero = [jax.device_put(
        np.zeros((C * z.shape[0], *z.shape[1:]), z.dtype), sh) for z in zero_outs]
    out = sharded(*concat_in, *concat_zero)   # warmup + compile
    jax.block_until_ready(out)
    best = None
    for _ in range(iters):
        t0 = time.perf_counter()
        out = sharded(*concat_in, *concat_zero)
        jax.block_until_ready(out)
        dt_ns = (time.perf_counter() - t0) * 1e9
        best = dt_ns if best is None else min(best, dt_ns)
    return int(best)


def kernel(feat, src, dst, W1, b1, W2, b2, W3, b3, Wlin, blin):
    global LAST_EXEC_NS, LAST_RESULTS
    feat = np.asarray(feat, np.float32)
    src = np.asarray(src, np.int32)
    dst = np.asarray(dst, np.int32)
    W1, b1 = np.asarray(W1, np.float32), np.asarray(b1, np.float32)
    W2, b2 = np.asarray(W2, np.float32), np.asarray(b2, np.float32)
    W3, b3 = np.asarray(W3, np.float32), np.asarray(b3, np.float32)
    Wlin, blin = np.asarray(Wlin, np.float32), np.asarray(blin, np.float32)

    TBMAX, BLOMAX, BHIMAX, blk_shared, common, percore = _host_prep(
        feat, src, dst, W1, b1, W2, b2, W3, b3)
    nc = _build(TBMAX, BLOMAX, BHIMAX, blk_shared)

    from concourse.bass_utils import run_bass_kernel_spmd
    in_maps = [dict(common, **percore[c]) for c in range(C)]
    res = run_bass_kernel_spmd(nc, in_maps, core_ids=list(range(C)))
    LAST_RESULTS = res
    if os.environ.get("KTIME"):
        LAST_EXEC_NS = _time_exec(nc, in_maps)

    total = np.zeros(D, np.float64)
    for c in range(C):
        total += res.results[c]["pooled"][:, 0].astype(np.float64)
    hg = (total / N).astype(np.float32)
    out = hg @ Wlin + blin
    return (1.0 / (1.0 + np.exp(-out.astype(np.float64)))).astype(np.float32)[None, :]
